# revision 4
# baseline (speedup 1.0000x reference)
"""Trainium2 Bass kernel for Llama-style GQA attention block (B=1, S=2048,
D=4096, 32 q heads / 8 kv heads, head_dim 128, neox RoPE, causal).

Sharding: tensor-parallel over kv heads across 8 NeuronCores. Core c gets
kv head c and q heads [4c, 4c+4). Each core computes a full [S, D] partial
of the output (o_proj row-parallel); host sums the 8 partials.

v3 (fused single pipeline): projections, attention, and o_proj emitted as
one interleaved PE instruction stream so the tensor engine never idles at
phase boundaries (the v2 A->B boundary cost ~5.5us idle + a HAM re-throttle).

Structure per S-slice sl of 512 (super-iteration):
  - projections run in TWO passes of 3 groups each (pass1 = {k, q0, q1},
    pass2 = {v, q2, q3}) so they hold only 3 PSUM banks; hT is re-streamed
    from HBM for pass2 (DMA has headroom, PSUM does not).
  - attention for q-slice qs=sl-1 runs HEAD-SERIAL (one PSUM bank for the
    PV accumulator), its score/PV matmuls popped between projection d-steps;
    exp latency hidden by a 2-deep score pipeline + proj-MM filler.
  - o_proj blocks of qs=sl-2 are popped as additional PE filler.
  PSUM budget: 3 (proj passes) + 2 (scores/rope/transpose/denominator,
  tag-shared) + 1 (PV accum) + 2 (o_proj) = 8 banks exactly.

Other changes vs v2: PV matmuls, exps and denominator adds are trimmed to
[dlo:] on diagonal tiles (no stale-byte priming needed; the causal staircase
mask shrinks to a single [128,128] 0/1 multiply), the denominator accumulator
is f32r from the start, and outT is stored bf16 (halves output DMA).
"""

import threading
from dataclasses import dataclass

import numpy as np


@dataclass(frozen=True)
class Cfg:
    S: int = 2048      # sequence length
    D: int = 4096      # hidden size
    HQ: int = 4        # q heads per core
    DH: int = 128      # head dim
    QSL: int = 512     # q-slice width (= matmul N)
    theta: float = 10000.0
    cores: int = 8


FULL = Cfg()
_DONE = object()


def build_nc(cfg: Cfg):
    import concourse.bass as bass  # noqa: F401
    import concourse.mybir as mybir
    import concourse.tile as tile
    from concourse import bacc

    F32 = mybir.dt.float32
    F32R = mybir.dt.float32r
    BF16 = mybir.dt.bfloat16

    S, D, HQ, DH, QSL = cfg.S, cfg.D, cfg.HQ, cfg.DH, cfg.QSL
    DT = D // 128          # d (contraction) tiles
    NQS = S // QSL         # q slices / S slices
    NDT = D // 128         # output D row-tiles (o_proj)
    KPS = QSL // 128       # k-tiles per slice
    scale = float(DH) ** -0.5
    Exp = mybir.ActivationFunctionType.Exp

    nc = bacc.Bacc("TRN2", target_bir_lowering=False, debug=False,
                   num_devices=cfg.cores)

    hT = nc.dram_tensor("hT", [D, S], BF16, kind="ExternalInput").ap()
    wq = nc.dram_tensor("wq", [D, HQ * DH], BF16, kind="ExternalInput").ap()
    wk = nc.dram_tensor("wk", [D, DH], BF16, kind="ExternalInput").ap()
    wv = nc.dram_tensor("wv", [D, DH], BF16, kind="ExternalInput").ap()
    wo = nc.dram_tensor("wo", [HQ * DH, D], BF16, kind="ExternalInput").ap()
    cosT = nc.dram_tensor("cosT", [DH, S], BF16, kind="ExternalInput").ap()
    sinT = nc.dram_tensor("sinT", [DH, S], BF16, kind="ExternalInput").ap()
    stair = nc.dram_tensor("stair", [128, 128], BF16,
                           kind="ExternalInput").ap()
    cstb = nc.dram_tensor("cstb", [128, 256], BF16, kind="ExternalInput").ap()
    onesf = nc.dram_tensor("onesf", [128, 128], F32R, kind="ExternalInput").ap()
    outT = nc.dram_tensor("outT", [D, S], BF16, kind="ExternalOutput").ap()

    with tile.TileContext(nc) as tc, \
            tc.tile_pool(name="main", bufs=1) as pm, \
            tc.tile_pool(name="hstream", bufs=8) as hp, \
            tc.tile_pool(name="expp", bufs=10) as ep, \
            tc.tile_pool(name="ropet", bufs=4) as rtp, \
            tc.tile_pool(name="accp", bufs=2) as ap_, \
            tc.tile_pool(name="dnp", bufs=2) as dp, \
            tc.tile_pool(name="ocp", bufs=6) as ocp, \
            tc.tile_pool(name="psA", bufs=3, space="PSUM") as psA, \
            tc.tile_pool(name="psS", bufs=2, space="PSUM") as psS, \
            tc.tile_pool(name="psO", bufs=1, space="PSUM") as psO, \
            tc.tile_pool(name="psC", bufs=2, space="PSUM") as psC:
        # long-lived SBUF tensors
        qT = [pm.tile([128, S], BF16, tag=f"qT{g}", name=f"qT{g}")
              for g in range(HQ)]
        kT = pm.tile([128, S], BF16, tag="kT")
        vT = pm.tile([128, S], BF16, tag="vT")
        v_all = pm.tile([128, S // 128, DH], BF16, tag="vall")
        o_attn = [pm.tile([128, S], BF16, tag=f"oT{g}", name=f"oT{g}")
                  for g in range(HQ)]
        cos_sb = pm.tile([128, S], BF16, tag="cos")
        sin_sb = pm.tile([128, S], BF16, tag="sin")
        stair_sb = pm.tile([128, 128], BF16, tag="stair")
        cst_sb = pm.tile([128, 256], BF16, tag="cstb")
        ones_sb = pm.tile([128, 128], F32R, tag="ones")
        wq_sb = pm.tile([128, DT, HQ * DH], BF16, tag="wq")
        wk_sb = pm.tile([128, DT, DH], BF16, tag="wk")
        wv_sb = pm.tile([128, DT, DH], BF16, tag="wv")
        wo_sb = pm.tile([128, HQ, D], BF16, tag="wo")

        rot_sb = cst_sb[:, 0:128]
        ident = cst_sb[:, 128:256]

        # ---- prologue DMAs (scalar ring; wk/wq first so MM 0 starts ~1us) ----
        wq_r = wq.rearrange("(t p) m -> p t m", p=128)
        wk_r = wk.rearrange("(t p) m -> p t m", p=128)
        wv_r = wv.rearrange("(t p) m -> p t m", p=128)
        # wk/wq feed pass1 immediately; wv only feeds pass2 (~21us in) and
        # the RoPE tables are first read at the end of pass1(0), so order:
        # all wk+wq chunks, tables, wv, masks.
        chunks = [(0, 1), (1, 3), (3, 7), (7, 15), (15, 24), (24, 32)]
        for c0, c1 in chunks:
            cs = slice(c0, c1)
            nc.scalar.dma_start(out=wk_sb[:, cs, :], in_=wk_r[:, cs, :])
            nc.scalar.dma_start(out=wq_sb[:, cs, :], in_=wq_r[:, cs, :])
        nc.scalar.dma_start(out=cst_sb, in_=cstb)
        nc.scalar.dma_start(out=cos_sb, in_=cosT)
        nc.scalar.dma_start(out=sin_sb, in_=sinT)
        for c0, c1 in chunks:
            cs = slice(c0, c1)
            nc.scalar.dma_start(out=wv_sb[:, cs, :], in_=wv_r[:, cs, :])
        nc.scalar.dma_start(out=stair_sb, in_=stair)
        nc.scalar.dma_start(out=ones_sb, in_=onesf)

        hT_r = hT.rearrange("(t p) s -> p t s", p=128)

        # ---------------- emitters ----------------
        def emit_rope(blk, ssl):
            """blk[:, ssl] = blk*cos + rot(blk)*sin, rot via PE matmul."""
            pr = psS.tile([128, QSL], F32, tag="ps", name="pr")
            nc.tensor.matmul(pr, rot_sb, blk[:, ssl], start=True, stop=True)
            rs = rtp.tile([128, QSL], BF16, tag="rs")
            nc.vector.tensor_mul(rs, pr, sin_sb[:, ssl])
            cc = rtp.tile([128, QSL], BF16, tag="rc")
            nc.vector.tensor_mul(cc, blk[:, ssl], cos_sb[:, ssl])
            nc.vector.tensor_add(blk[:, ssl], cc, rs)

        def emit_vtr(t):
            """v_all[:, t, :] = vT[:, t*128:(t+1)*128].T (natural layout)."""
            pt = psS.tile([128, 128], BF16, tag="ps", name="pt")
            nc.tensor.transpose(pt, vT[:, t * 128:(t + 1) * 128], ident)
            nc.scalar.copy(v_all[:, t, :], pt)

        def emit_block(qs, Dt):
            """One o_proj [128, QSL] output block for q-slice qs."""
            qsl_ = slice(qs * QSL, (qs + 1) * QSL)
            pf = psC.tile([128, QSL], F32, tag="psC", name="psC")
            for g in range(HQ):
                nc.tensor.matmul(
                    pf, wo_sb[:, g, Dt * 128:(Dt + 1) * 128],
                    o_attn[g][:, qsl_],
                    start=(g == 0), stop=(g == HQ - 1))
            oc = ocp.tile([128, QSL], BF16, tag="oc")
            nc.scalar.copy(oc, pf)
            nc.sync.dma_start(out=outT[Dt * 128:(Dt + 1) * 128, qsl_], in_=oc)

        def oproj_stream(qs, dts):
            for Dt in dts:
                emit_block(qs, Dt)
                yield

        def attn_stream(qs):
            """Head-serial flash attention for q-slice qs. One PV PSUM bank.
            Yields after each PE-op-sized step."""
            qb = qs * QSL
            qsl_ = slice(qb, qb + QSL)
            nkt = (qs + 1) * KPS
            for g in range(HQ):
                po = psO.tile([128, QSL], F32, tag="psO", name="po")
                acc = ap_.tile([128, QSL], F32R, tag="acc", name="acc")
                ex = [None] * nkt

                def emit_s(t):
                    delta = t * 128 - qb
                    dlo = max(0, delta)
                    ps = psS.tile([128, QSL], F32, tag="ps", name="ps")
                    nc.tensor.matmul(ps[:, dlo:],
                                     kT[:, t * 128:(t + 1) * 128],
                                     qT[g][:, qb + dlo:qb + QSL],
                                     start=True, stop=True)
                    e = ep.tile([128, QSL], BF16, tag="exp", name="ex")
                    nc.scalar.activation(e[:, dlo:], ps[:, dlo:], Exp,
                                         bias=0.0, scale=scale)
                    if delta >= 0:
                        # causal staircase on the 128 diagonal columns
                        nc.vector.tensor_mul(e[:, dlo:dlo + 128],
                                             e[:, dlo:dlo + 128], stair_sb)
                    ex[t] = e

                def emit_p(t):
                    dlo = max(0, t * 128 - qb)
                    e = ex[t]
                    nc.tensor.matmul(po[:, dlo:], v_all[:, t, :], e[:, dlo:],
                                     start=(t == 0), stop=(t == nkt - 1))
                    if t == 0:
                        nc.vector.tensor_copy(acc, e)
                    else:
                        nc.vector.tensor_add(acc[:, dlo:], acc[:, dlo:],
                                             e[:, dlo:])
                    ex[t] = None

                emit_s(0)
                yield
                for t in range(1, nkt):
                    emit_s(t)
                    yield
                    emit_p(t - 1)
                    yield
                emit_p(nkt - 1)
                yield
                # denominator (partition-sum via ones matmul) + normalize
                pd = psS.tile([128, QSL], F32, tag="ps", name="pd")
                nc.tensor.matmul(pd, ones_sb, acc, start=True, stop=True)
                dn = dp.tile([128, QSL], F32, tag="dn")
                nc.vector.reciprocal_approx_fast(dn, pd)
                nc.vector.tensor_mul(o_attn[g][:, qsl_], po, dn)
                yield

        def attn_count(qs):
            return HQ * (2 * (qs + 1) * KPS + 1)

        def merged(streams):
            """Proportionally interleave step-generators (Bresenham)."""
            counts = [n for _, n in streams]
            total = sum(counts)
            accs = [0.0] * len(streams)
            live = [g for g, _ in streams]
            for _ in range(total):
                for j in range(len(streams)):
                    accs[j] += counts[j]
                j = max(range(len(streams)), key=lambda k: accs[k])
                accs[j] -= total
                if next(live[j], _DONE) is not _DONE:
                    yield

        def gen_of(closures):
            for c in closures:
                c()
                yield

        # group selectors: (weight_tile_fn, dst, copy_engine)
        def pass_groups(half):
            if half == 0:
                return [
                    (lambda d: wk_sb[:, d, :], kT, 's'),
                    (lambda d: wq_sb[:, d, 0 * DH:1 * DH], qT[0], 'v'),
                    (lambda d: wq_sb[:, d, 1 * DH:2 * DH], qT[1], 's'),
                ]
            return [
                (lambda d: wv_sb[:, d, :], vT, 'v'),
                (lambda d: wq_sb[:, d, 2 * DH:3 * DH], qT[2], 's'),
                (lambda d: wq_sb[:, d, 3 * DH:4 * DH], qT[3], 'v'),
            ]

        def super_iter(sl, extras_it, n_extras):
            ssl = slice(sl * QSL, (sl + 1) * QSL)
            per_step = n_extras / 64.0
            budget = 0.0

            def pop():
                nonlocal budget
                budget += per_step
                while budget >= 1.0:
                    budget -= 1.0
                    if next(extras_it, _DONE) is _DONE:
                        budget = 0.0
                        return

            for half in range(2):
                groups = pass_groups(half)
                pss = [psA.tile([128, QSL], F32, tag="psA",
                                name=f"psA_{sl}_{half}_{i}")
                       for i in range(3)]
                for d0 in range(0, DT, 2):
                    ht2 = hp.tile([128, 2, QSL], BF16, tag="ht")
                    nc.sync.dma_start(out=ht2, in_=hT_r[:, d0:d0 + 2, ssl])
                    for j in range(2):
                        d = d0 + j
                        for gi, (wsel, _, _) in enumerate(groups):
                            nc.tensor.matmul(pss[gi], wsel(d), ht2[:, j, :],
                                             start=(d == 0),
                                             stop=(d == DT - 1))
                        pop()
                for gi, (_, dst, eng) in enumerate(groups):
                    if eng == 's':
                        nc.scalar.copy(dst[:, ssl], pss[gi])
                    else:
                        nc.vector.tensor_copy(dst[:, ssl], pss[gi])
                if half == 0:
                    # k/q0/q1 of this slice exist now; RoPE them so the
                    # next super-iteration's attention can use them.
                    for blk in (kT, qT[0], qT[1]):
                        emit_rope(blk, ssl)

        # ---------------- main pipeline ----------------
        for sl in range(NQS):
            pre = []
            streams = []
            if sl >= 1:
                pssl = slice((sl - 1) * QSL, sl * QSL)
                pre.append(lambda s_=pssl: emit_rope(qT[2], s_))
                pre.append(lambda s_=pssl: emit_rope(qT[3], s_))
                for tt in range(KPS):
                    pre.append(lambda t_=(sl - 1) * KPS + tt: emit_vtr(t_))
                streams.append((attn_stream(sl - 1), attn_count(sl - 1)))
            if sl >= 2:
                streams.append((oproj_stream(sl - 2, range(NDT)), NDT))
            extras = _chain(gen_of(pre), merged(streams))
            n_extras = len(pre) + sum(n for _, n in streams)
            super_iter(sl, extras, n_extras)
            for _ in extras:   # drain leftovers (rounding)
                pass
            if sl == 0:
                # wo needed from first o_proj block (~super-iter 2);
                # deferred here to keep it off the critical startup DMAs.
                nc.scalar.dma_start(
                    out=wo_sb, in_=wo.rearrange("(t p) n -> p t n", p=128))

        # ---------------- tail: attn(3) + o_proj(2,3) ----------------
        lsl = slice((NQS - 1) * QSL, NQS * QSL)
        emit_rope(qT[2], lsl)
        emit_rope(qT[3], lsl)
        for tt in range(KPS):
            emit_vtr((NQS - 1) * KPS + tt)
        reserve = 4
        tail = merged([(attn_stream(NQS - 1), attn_count(NQS - 1)),
                       (oproj_stream(NQS - 2, range(NDT - reserve)),
                        NDT - reserve)])
        for _ in tail:
            pass
        # reserve blocks cover the last head's denominator-chain latency
        for Dt in range(NDT - reserve, NDT):
            emit_block(NQS - 2, Dt)
        for Dt in range(NDT):
            emit_block(NQS - 1, Dt)

    nc.compile()
    return nc


def _chain(*its):
    for it in its:
        yield from it


def make_tables(cfg: Cfg, position_ids: np.ndarray):
    """cosT/sinT [128, S]: row d holds cos/sin(pos * invfreq[d % 64])."""
    half = cfg.DH // 2
    inv = 1.0 / (cfg.theta ** (np.arange(half, dtype=np.float64) * 2.0 / cfg.DH))
    pos = np.asarray(position_ids).reshape(-1).astype(np.float64)  # [S]
    ang = inv[:, None] * pos[None, :]                              # [64, S]
    cosT = np.concatenate([np.cos(ang), np.cos(ang)], 0)
    sinT = np.concatenate([np.sin(ang), np.sin(ang)], 0)
    return cosT, sinT


def make_cst(cfg: Cfg):
    """[128, 256] cols 0-127: rotate-half stationary matrix (out = M^T @ x,
    out[:64] = -x[64:], out[64:] = x[:64]); cols 128-255: identity."""
    half = cfg.DH // 2
    m = np.zeros((128, 256), np.float64)
    for i in range(half):
        m[i + half, i] = -1.0
        m[i, i + half] = 1.0
    m[:, 128:256] = np.eye(128)
    return m


def make_stair():
    """0/1 staircase [128,128]: col j, row p -> 1 if j >= p else 0.
    Applied to the 128 diagonal columns [dlo, dlo+128) of each diagonal
    score tile (delta >= 0), where col j-dlo vs row p encodes k <= q."""
    j = np.arange(128)[None, :]
    p = np.arange(128)[:, None]
    return (j >= p).astype(np.float64)


_cache = threading.Lock()
_nc_full = None


def _get_nc():
    global _nc_full
    with _cache:
        if _nc_full is None:
            _nc_full = build_nc(FULL)
    return _nc_full


def core_inputs(cfg: Cfg, c: int, position_ids, hidden_states, Wq, Wk, Wv, Wo):
    """Build the per-core input map (numpy, bf16 operands) for core c."""
    from ml_dtypes import bfloat16

    def bf(x):
        return np.ascontiguousarray(np.asarray(x).astype(bfloat16))

    S, D, HQ, DH = cfg.S, cfg.D, cfg.HQ, cfg.DH
    hT = np.asarray(hidden_states, dtype=np.float32).reshape(S, D).T
    cosT, sinT = make_tables(cfg, position_ids)
    qc = slice(c * HQ * DH, (c + 1) * HQ * DH)
    kc = slice(c * DH, (c + 1) * DH)
    return {
        "hT": bf(hT),
        "wq": bf(np.asarray(Wq, np.float32)[:, qc]),
        "wk": bf(np.asarray(Wk, np.float32)[:, kc]),
        "wv": bf(np.asarray(Wv, np.float32)[:, kc]),
        "wo": bf(np.asarray(Wo, np.float32)[qc, :]),
        "cosT": bf(cosT),
        "sinT": bf(sinT),
        "stair": bf(make_stair()),
        "cstb": bf(make_cst(cfg)),
        "onesf": np.ones((128, 128), np.float32),
    }


def kernel(position_ids, hidden_states, Wq, Wk, Wv, Wo, _trace=False):
    from concourse.bass_utils import run_bass_kernel_spmd

    cfg = FULL
    nc = _get_nc()
    args = (position_ids, hidden_states, Wq, Wk, Wv, Wo)
    in_maps = [core_inputs(cfg, c, *args) for c in range(cfg.cores)]
    res = run_bass_kernel_spmd(nc, in_maps, core_ids=list(range(cfg.cores)),
                               trace=_trace)
    out = np.zeros((cfg.S, cfg.D), np.float64)
    for c in range(cfg.cores):
        out += res.results[c]["outT"].T.astype(np.float64)
    ret = out.astype(np.float32).reshape(1, cfg.S, cfg.D)
    if _trace:
        return ret, res
    return ret


# revision 10
# speedup vs baseline: 1.0306x; 1.0306x over previous
"""Trainium2 Bass kernel for Llama-style GQA attention block (B=1, S=2048,
D=4096, 32 q heads / 8 kv heads, head_dim 128, neox RoPE, causal).

Sharding: tensor-parallel over kv heads across 8 NeuronCores. Core c gets
kv head c and q heads [4c, 4c+4). Each core computes a full [S, D] partial
of the output (o_proj row-parallel); host sums the 8 partials.

v3 (fused single pipeline): projections, attention, and o_proj emitted as
one interleaved PE instruction stream so the tensor engine never idles at
phase boundaries (the v2 A->B boundary cost ~5.5us idle + a HAM re-throttle).

Structure per S-slice sl of 512 (super-iteration):
  - projections run in TWO passes of 3 groups each (pass1 = {k, q0, q1},
    pass2 = {v, q2, q3}) so they hold only 3 PSUM banks; hT is re-streamed
    from HBM for pass2 (DMA has headroom, PSUM does not).
  - attention for q-slice qs=sl-1 runs HEAD-SERIAL (one PSUM bank for the
    PV accumulator), its score/PV matmuls popped between projection d-steps;
    exp latency hidden by a 2-deep score pipeline + proj-MM filler.
  - o_proj blocks of qs=sl-2 are popped as additional PE filler.
  PSUM budget: 3 (proj passes) + 2 (scores/rope/transpose/denominator,
  tag-shared) + 1 (PV accum) + 2 (o_proj) = 8 banks exactly.

Other changes vs v2: PV matmuls, exps and denominator adds are trimmed to
[dlo:] on diagonal tiles (no stale-byte priming needed; the causal staircase
mask shrinks to a single [128,128] 0/1 multiply), the denominator accumulator
is f32r from the start, and outT is stored bf16 (halves output DMA).
"""

import threading
from dataclasses import dataclass

import numpy as np


@dataclass(frozen=True)
class Cfg:
    S: int = 2048      # sequence length
    D: int = 4096      # hidden size
    HQ: int = 4        # q heads per core
    DH: int = 128      # head dim
    QSL: int = 512     # q-slice width (= matmul N)
    theta: float = 10000.0
    cores: int = 8


FULL = Cfg()
_DONE = object()


def build_nc(cfg: Cfg):
    import concourse.bass as bass  # noqa: F401
    import concourse.mybir as mybir
    import concourse.tile as tile
    from concourse import bacc

    F32 = mybir.dt.float32
    F32R = mybir.dt.float32r
    BF16 = mybir.dt.bfloat16

    S, D, HQ, DH, QSL = cfg.S, cfg.D, cfg.HQ, cfg.DH, cfg.QSL
    DT = D // 128          # d (contraction) tiles
    NQS = S // QSL         # q slices / S slices
    NDT = D // 128         # output D row-tiles (o_proj)
    KPS = QSL // 128       # k-tiles per slice
    scale = float(DH) ** -0.5
    Exp = mybir.ActivationFunctionType.Exp

    nc = bacc.Bacc("TRN2", target_bir_lowering=False, debug=False,
                   num_devices=cfg.cores)

    hT = nc.dram_tensor("hT", [D, S], BF16, kind="ExternalInput").ap()
    wq = nc.dram_tensor("wq", [D, HQ * DH], BF16, kind="ExternalInput").ap()
    wk = nc.dram_tensor("wk", [D, DH], BF16, kind="ExternalInput").ap()
    wv = nc.dram_tensor("wv", [D, DH], BF16, kind="ExternalInput").ap()
    wo = nc.dram_tensor("wo", [HQ * DH, D], BF16, kind="ExternalInput").ap()
    cosT = nc.dram_tensor("cosT", [DH, S], BF16, kind="ExternalInput").ap()
    sinT = nc.dram_tensor("sinT", [DH, S], BF16, kind="ExternalInput").ap()
    stair = nc.dram_tensor("stair", [128, 128], BF16,
                           kind="ExternalInput").ap()
    cstb = nc.dram_tensor("cstb", [128, 256], BF16, kind="ExternalInput").ap()
    onesf = nc.dram_tensor("onesf", [128, 128], F32R, kind="ExternalInput").ap()
    outT = nc.dram_tensor("outT", [D, S], BF16, kind="ExternalOutput").ap()

    with tile.TileContext(nc) as tc, \
            tc.tile_pool(name="main", bufs=1) as pm, \
            tc.tile_pool(name="hstream", bufs=5) as hp, \
            tc.tile_pool(name="expp", bufs=10) as ep, \
            tc.tile_pool(name="ropet", bufs=4) as rtp, \
            tc.tile_pool(name="accp", bufs=2) as ap_, \
            tc.tile_pool(name="dnp", bufs=2) as dp, \
            tc.tile_pool(name="ocp", bufs=6) as ocp, \
            tc.tile_pool(name="psA", bufs=3, space="PSUM") as psA, \
            tc.tile_pool(name="psS", bufs=2, space="PSUM") as psS, \
            tc.tile_pool(name="psO", bufs=1, space="PSUM") as psO, \
            tc.tile_pool(name="psC", bufs=2, space="PSUM") as psC:
        # long-lived SBUF tensors
        qT = [pm.tile([128, S], BF16, tag=f"qT{g}", name=f"qT{g}")
              for g in range(HQ)]
        kT = pm.tile([128, S], BF16, tag="kT")
        vT = pm.tile([128, S], BF16, tag="vT")
        v_all = pm.tile([128, S // 128, DH], BF16, tag="vall")
        o_attn = [pm.tile([128, S], BF16, tag=f"oT{g}", name=f"oT{g}")
                  for g in range(HQ)]
        cos_sb = pm.tile([128, S], BF16, tag="cos")
        sin_sb = pm.tile([128, S], BF16, tag="sin")
        stair_sb = pm.tile([128, 128], BF16, tag="stair")
        cst_sb = pm.tile([128, 256], BF16, tag="cstb")
        ones_sb = pm.tile([128, 128], F32R, tag="ones")
        wq_sb = pm.tile([128, DT, HQ * DH], BF16, tag="wq")
        wk_sb = pm.tile([128, DT, DH], BF16, tag="wk")
        wv_sb = pm.tile([128, DT, DH], BF16, tag="wv")
        wo_sb = pm.tile([128, HQ, D], BF16, tag="wo")

        rot_sb = cst_sb[:, 0:128]
        ident = cst_sb[:, 128:256]

        # ---- prologue DMAs (scalar ring; wk/wq first so MM 0 starts ~1us) ----
        wq_r = wq.rearrange("(t p) m -> p t m", p=128)
        wk_r = wk.rearrange("(t p) m -> p t m", p=128)
        wv_r = wv.rearrange("(t p) m -> p t m", p=128)
        # wk/wq feed pass1 immediately; wv only feeds pass2 (~21us in) and
        # the RoPE tables are first read at the end of pass1(0), so order:
        # all wk+wq chunks, tables, wv, masks.
        chunks = [(0, 1), (1, 3), (3, 7), (7, 15), (15, 24), (24, 32)]
        for c0, c1 in chunks:
            cs = slice(c0, c1)
            nc.scalar.dma_start(out=wk_sb[:, cs, :], in_=wk_r[:, cs, :])
            nc.scalar.dma_start(out=wq_sb[:, cs, :], in_=wq_r[:, cs, :])
        nc.scalar.dma_start(out=cst_sb, in_=cstb)
        nc.scalar.dma_start(out=cos_sb, in_=cosT)
        nc.scalar.dma_start(out=sin_sb, in_=sinT)
        for c0, c1 in chunks:
            cs = slice(c0, c1)
            nc.scalar.dma_start(out=wv_sb[:, cs, :], in_=wv_r[:, cs, :])
        nc.scalar.dma_start(out=stair_sb, in_=stair)
        nc.scalar.dma_start(out=ones_sb, in_=onesf)

        hT_r = hT.rearrange("(t p) s -> p t s", p=128)

        # ---------------- emitters ----------------
        def emit_rope(blk, ssl):
            """blk[:, ssl] = blk*cos + rot(blk)*sin, rot via PE matmul."""
            pr = psS.tile([128, QSL], F32, tag="ps", name="pr")
            nc.tensor.matmul(pr, rot_sb, blk[:, ssl], start=True, stop=True)
            rs = rtp.tile([128, QSL], BF16, tag="rs")
            nc.vector.tensor_mul(rs, pr, sin_sb[:, ssl])
            cc = rtp.tile([128, QSL], BF16, tag="rc")
            nc.vector.tensor_mul(cc, blk[:, ssl], cos_sb[:, ssl])
            nc.vector.tensor_add(blk[:, ssl], cc, rs)

        def emit_vtr(t):
            """v_all[:, t, :] = vT[:, t*128:(t+1)*128].T (natural layout)."""
            pt = psS.tile([128, 128], BF16, tag="ps", name="pt")
            nc.tensor.transpose(pt, vT[:, t * 128:(t + 1) * 128], ident)
            nc.scalar.copy(v_all[:, t, :], pt)

        def emit_block(qs, Dt):
            """One o_proj [128, QSL] output block for q-slice qs."""
            qsl_ = slice(qs * QSL, (qs + 1) * QSL)
            pf = psC.tile([128, QSL], F32, tag="psC", name="psC")
            for g in range(HQ):
                nc.tensor.matmul(
                    pf, wo_sb[:, g, Dt * 128:(Dt + 1) * 128],
                    o_attn[g][:, qsl_],
                    start=(g == 0), stop=(g == HQ - 1))
            oc = ocp.tile([128, QSL], BF16, tag="oc")
            nc.scalar.copy(oc, pf)
            nc.sync.dma_start(out=outT[Dt * 128:(Dt + 1) * 128, qsl_], in_=oc)

        def oproj_stream(qs, dts):
            for Dt in dts:
                emit_block(qs, Dt)
                yield

        def attn_stream(qs):
            """Head-serial flash attention for q-slice qs. One PV PSUM bank.
            Yields after each PE-op-sized step."""
            qb = qs * QSL
            qsl_ = slice(qb, qb + QSL)
            nkt = (qs + 1) * KPS
            for g in range(HQ):
                po = psO.tile([128, QSL], F32, tag="psO", name="po")
                acc = ap_.tile([128, QSL], F32R, tag="acc", name="acc")
                ex = [None] * nkt

                def emit_s(t):
                    delta = t * 128 - qb
                    dlo = max(0, delta)
                    ps = psS.tile([128, QSL], F32, tag="ps", name="ps")
                    nc.tensor.matmul(ps[:, dlo:],
                                     kT[:, t * 128:(t + 1) * 128],
                                     qT[g][:, qb + dlo:qb + QSL],
                                     start=True, stop=True)
                    e = ep.tile([128, QSL], BF16, tag="exp", name="ex")
                    nc.scalar.activation(e[:, dlo:], ps[:, dlo:], Exp,
                                         bias=0.0, scale=scale)
                    if delta >= 0:
                        # causal staircase on the 128 diagonal columns
                        nc.vector.tensor_mul(e[:, dlo:dlo + 128],
                                             e[:, dlo:dlo + 128], stair_sb)
                    ex[t] = e

                def emit_p(t):
                    dlo = max(0, t * 128 - qb)
                    e = ex[t]
                    nc.tensor.matmul(po[:, dlo:], v_all[:, t, :], e[:, dlo:],
                                     start=(t == 0), stop=(t == nkt - 1))
                    if t == 0:
                        nc.vector.tensor_copy(acc, e)
                    else:
                        nc.vector.tensor_add(acc[:, dlo:], acc[:, dlo:],
                                             e[:, dlo:])
                    ex[t] = None

                emit_s(0)
                yield
                for t in range(1, nkt):
                    emit_s(t)
                    yield
                    emit_p(t - 1)
                    yield
                emit_p(nkt - 1)
                yield
                # denominator (partition-sum via ones matmul) + normalize
                pd = psS.tile([128, QSL], F32, tag="ps", name="pd")
                nc.tensor.matmul(pd, ones_sb, acc, start=True, stop=True)
                dn = dp.tile([128, QSL], F32, tag="dn")
                nc.vector.reciprocal_approx_fast(dn, pd)
                nc.vector.tensor_mul(o_attn[g][:, qsl_], po, dn)
                yield

        def attn_count(qs):
            return HQ * (2 * (qs + 1) * KPS + 1)

        def merged(streams):
            """Proportionally interleave step-generators (Bresenham)."""
            counts = [n for _, n in streams]
            total = sum(counts)
            accs = [0.0] * len(streams)
            live = [g for g, _ in streams]
            for _ in range(total):
                for j in range(len(streams)):
                    accs[j] += counts[j]
                j = max(range(len(streams)), key=lambda k: accs[k])
                accs[j] -= total
                if next(live[j], _DONE) is not _DONE:
                    yield

        def gen_of(closures):
            for c in closures:
                c()
                yield

        # group selectors: (weight_tile_fn, dst, copy_engine)
        def pass_groups(half):
            if half == 0:
                return [
                    (lambda d: wk_sb[:, d, :], kT, 's'),
                    (lambda d: wq_sb[:, d, 0 * DH:1 * DH], qT[0], 'v'),
                    (lambda d: wq_sb[:, d, 1 * DH:2 * DH], qT[1], 's'),
                ]
            return [
                (lambda d: wv_sb[:, d, :], vT, 'v'),
                (lambda d: wq_sb[:, d, 2 * DH:3 * DH], qT[2], 's'),
                (lambda d: wq_sb[:, d, 3 * DH:4 * DH], qT[3], 'v'),
            ]

        _pref = [None]  # prefetched sub0 hT tile for the next slice

        def super_iter(sl, extras_it, n_extras):
            ssl = slice(sl * QSL, (sl + 1) * QSL)
            per_step = n_extras / 64.0
            budget = 0.0

            def pop():
                nonlocal budget
                budget += per_step
                while budget >= 1.0:
                    budget -= 1.0
                    if next(extras_it, _DONE) is _DONE:
                        budget = 0.0
                        return

            # hT slice staged once in SBUF sub-tiles; pass2 re-reads the
            # resident tiles (no HBM re-stream). Sub-tile DMAs are emitted
            # just-in-time so prefetch doesn't steal startup DMA bandwidth;
            # sub0 of the NEXT slice is prefetched during pass2 so the
            # super-iteration boundary doesn't stall on its DMA.
            hts = [None] * (DT // 8)
            if _pref[0] is not None:
                hts[0] = _pref[0]
                _pref[0] = None
            for half in range(2):
                groups = pass_groups(half)
                pss = [psA.tile([128, QSL], F32, tag="psA",
                                name=f"psA_{sl}_{half}_{i}")
                       for i in range(3)]
                for d0 in range(0, DT, 2):
                    if half == 0 and d0 % 8 == 0 and hts[d0 // 8] is None:
                        b = d0 // 8
                        hts[b] = hp.tile([128, 8, QSL], BF16, tag="ht",
                                         name=f"ht_{sl}_{b}")
                        nc.sync.dma_start(out=hts[b],
                                          in_=hT_r[:, b * 8:(b + 1) * 8, ssl])
                    if half == 1 and d0 == 2 and sl + 1 < NQS:
                        nsl = slice((sl + 1) * QSL, (sl + 2) * QSL)
                        pt_ = hp.tile([128, 8, QSL], BF16, tag="ht",
                                      name=f"ht_{sl + 1}_0")
                        nc.sync.dma_start(out=pt_, in_=hT_r[:, 0:8, nsl])
                        _pref[0] = pt_
                    for j in range(2):
                        d = d0 + j
                        for gi, (wsel, _, _) in enumerate(groups):
                            nc.tensor.matmul(pss[gi], wsel(d),
                                             hts[d // 8][:, d % 8, :],
                                             start=(d == 0),
                                             stop=(d == DT - 1))
                        pop()
                for gi, (_, dst, eng) in enumerate(groups):
                    if eng == 's':
                        nc.scalar.copy(dst[:, ssl], pss[gi])
                    else:
                        nc.vector.tensor_copy(dst[:, ssl], pss[gi])
                if half == 0:
                    # k/q0/q1 of this slice exist now; RoPE them so the
                    # next super-iteration's attention can use them.
                    for blk in (kT, qT[0], qT[1]):
                        emit_rope(blk, ssl)

        # ---------------- main pipeline ----------------
        for sl in range(NQS):
            pre = []
            streams = []
            if sl >= 1:
                pssl = slice((sl - 1) * QSL, sl * QSL)
                pre.append(lambda s_=pssl: emit_rope(qT[2], s_))
                pre.append(lambda s_=pssl: emit_rope(qT[3], s_))
                for tt in range(KPS):
                    pre.append(lambda t_=(sl - 1) * KPS + tt: emit_vtr(t_))
                streams.append((attn_stream(sl - 1), attn_count(sl - 1)))
            if sl >= 2:
                streams.append((oproj_stream(sl - 2, range(NDT)), NDT))
            extras = _chain(gen_of(pre), merged(streams))
            n_extras = len(pre) + sum(n for _, n in streams)
            super_iter(sl, extras, n_extras)
            for _ in extras:   # drain leftovers (rounding)
                pass
            if sl == 0:
                # wo needed from first o_proj block (~super-iter 2);
                # deferred + chunked to keep it off the critical startup DMAs.
                wo_r = wo.rearrange("(t p) n -> p t n", p=128)
                for t_ in range(HQ):
                    nc.scalar.dma_start(out=wo_sb[:, t_, :],
                                        in_=wo_r[:, t_, :])

        # ---------------- tail: attn(3) + o_proj(2,3) ----------------
        lsl = slice((NQS - 1) * QSL, NQS * QSL)
        emit_rope(qT[2], lsl)
        emit_rope(qT[3], lsl)
        for tt in range(KPS):
            emit_vtr((NQS - 1) * KPS + tt)
        reserve = 4
        tail = merged([(attn_stream(NQS - 1), attn_count(NQS - 1)),
                       (oproj_stream(NQS - 2, range(NDT - reserve)),
                        NDT - reserve)])
        for _ in tail:
            pass
        # reserve blocks cover the last head's denominator-chain latency
        for Dt in range(NDT - reserve, NDT):
            emit_block(NQS - 2, Dt)
        for Dt in range(NDT):
            emit_block(NQS - 1, Dt)

    nc.compile()
    return nc


def _chain(*its):
    for it in its:
        yield from it


def make_tables(cfg: Cfg, position_ids: np.ndarray):
    """cosT/sinT [128, S]: row d holds cos/sin(pos * invfreq[d % 64])."""
    half = cfg.DH // 2
    inv = 1.0 / (cfg.theta ** (np.arange(half, dtype=np.float64) * 2.0 / cfg.DH))
    pos = np.asarray(position_ids).reshape(-1).astype(np.float64)  # [S]
    ang = inv[:, None] * pos[None, :]                              # [64, S]
    cosT = np.concatenate([np.cos(ang), np.cos(ang)], 0)
    sinT = np.concatenate([np.sin(ang), np.sin(ang)], 0)
    return cosT, sinT


def make_cst(cfg: Cfg):
    """[128, 256] cols 0-127: rotate-half stationary matrix (out = M^T @ x,
    out[:64] = -x[64:], out[64:] = x[:64]); cols 128-255: identity."""
    half = cfg.DH // 2
    m = np.zeros((128, 256), np.float64)
    for i in range(half):
        m[i + half, i] = -1.0
        m[i, i + half] = 1.0
    m[:, 128:256] = np.eye(128)
    return m


def make_stair():
    """0/1 staircase [128,128]: col j, row p -> 1 if j >= p else 0.
    Applied to the 128 diagonal columns [dlo, dlo+128) of each diagonal
    score tile (delta >= 0), where col j-dlo vs row p encodes k <= q."""
    j = np.arange(128)[None, :]
    p = np.arange(128)[:, None]
    return (j >= p).astype(np.float64)


_cache = threading.Lock()
_nc_full = None


def _get_nc():
    global _nc_full
    with _cache:
        if _nc_full is None:
            _nc_full = build_nc(FULL)
    return _nc_full


def core_inputs(cfg: Cfg, c: int, position_ids, hidden_states, Wq, Wk, Wv, Wo):
    """Build the per-core input map (numpy, bf16 operands) for core c."""
    from ml_dtypes import bfloat16

    def bf(x):
        return np.ascontiguousarray(np.asarray(x).astype(bfloat16))

    S, D, HQ, DH = cfg.S, cfg.D, cfg.HQ, cfg.DH
    hT = np.asarray(hidden_states, dtype=np.float32).reshape(S, D).T
    cosT, sinT = make_tables(cfg, position_ids)
    qc = slice(c * HQ * DH, (c + 1) * HQ * DH)
    kc = slice(c * DH, (c + 1) * DH)
    return {
        "hT": bf(hT),
        "wq": bf(np.asarray(Wq, np.float32)[:, qc]),
        "wk": bf(np.asarray(Wk, np.float32)[:, kc]),
        "wv": bf(np.asarray(Wv, np.float32)[:, kc]),
        "wo": bf(np.asarray(Wo, np.float32)[qc, :]),
        "cosT": bf(cosT),
        "sinT": bf(sinT),
        "stair": bf(make_stair()),
        "cstb": bf(make_cst(cfg)),
        "onesf": np.ones((128, 128), np.float32),
    }


def kernel(position_ids, hidden_states, Wq, Wk, Wv, Wo, _trace=False):
    from concourse.bass_utils import run_bass_kernel_spmd

    cfg = FULL
    nc = _get_nc()
    args = (position_ids, hidden_states, Wq, Wk, Wv, Wo)
    in_maps = [core_inputs(cfg, c, *args) for c in range(cfg.cores)]
    res = run_bass_kernel_spmd(nc, in_maps, core_ids=list(range(cfg.cores)),
                               trace=_trace)
    out = np.zeros((cfg.S, cfg.D), np.float64)
    for c in range(cfg.cores):
        out += res.results[c]["outT"].T.astype(np.float64)
    ret = out.astype(np.float32).reshape(1, cfg.S, cfg.D)
    if _trace:
        return ret, res
    return ret


# revision 18
# speedup vs baseline: 1.0392x; 1.0083x over previous
"""Trainium2 Bass kernel for Llama-style GQA attention block (B=1, S=2048,
D=4096, 32 q heads / 8 kv heads, head_dim 128, neox RoPE, causal).

Sharding: tensor-parallel over kv heads across 8 NeuronCores. Core c gets
kv head c and q heads [4c, 4c+4). Each core computes a full [S, D] partial
of the output (o_proj row-parallel); host sums the 8 partials.

v3 (fused single pipeline): projections, attention, and o_proj emitted as
one interleaved PE instruction stream so the tensor engine never idles at
phase boundaries (the v2 A->B boundary cost ~5.5us idle + a HAM re-throttle).

Structure per S-slice sl of 512 (super-iteration):
  - projections run in TWO passes of 3 groups each (pass1 = {k, q0, q1},
    pass2 = {v, q2, q3}) so they hold only 3 PSUM banks; hT is re-streamed
    from HBM for pass2 (DMA has headroom, PSUM does not).
  - attention for q-slice qs=sl-1 runs HEAD-SERIAL (one PSUM bank for the
    PV accumulator), its score/PV matmuls popped between projection d-steps;
    exp latency hidden by a 2-deep score pipeline + proj-MM filler.
  - o_proj blocks of qs=sl-2 are popped as additional PE filler.
  PSUM budget: 3 (proj passes) + 2 (scores/rope/transpose/denominator,
  tag-shared) + 1 (PV accum) + 2 (o_proj) = 8 banks exactly.

Other changes vs v2: PV matmuls, exps and denominator adds are trimmed to
[dlo:] on diagonal tiles (no stale-byte priming needed; the causal staircase
mask shrinks to a single [128,128] 0/1 multiply), the denominator accumulator
is f32r from the start, and outT is stored bf16 (halves output DMA).
"""

import threading
from dataclasses import dataclass

import numpy as np


@dataclass(frozen=True)
class Cfg:
    S: int = 2048      # sequence length
    D: int = 4096      # hidden size
    HQ: int = 4        # q heads per core
    DH: int = 128      # head dim
    QSL: int = 512     # q-slice width (= matmul N)
    theta: float = 10000.0
    cores: int = 8


FULL = Cfg()
_DONE = object()


def build_nc(cfg: Cfg):
    import concourse.bass as bass  # noqa: F401
    import concourse.mybir as mybir
    import concourse.tile as tile
    from concourse import bacc

    F32 = mybir.dt.float32
    F32R = mybir.dt.float32r
    BF16 = mybir.dt.bfloat16

    S, D, HQ, DH, QSL = cfg.S, cfg.D, cfg.HQ, cfg.DH, cfg.QSL
    DT = D // 128          # d (contraction) tiles
    NQS = S // QSL         # q slices / S slices
    NDT = D // 128         # output D row-tiles (o_proj)
    KPS = QSL // 128       # k-tiles per slice
    scale = float(DH) ** -0.5
    Exp = mybir.ActivationFunctionType.Exp

    nc = bacc.Bacc("TRN2", target_bir_lowering=False, debug=False,
                   num_devices=cfg.cores)

    hT = nc.dram_tensor("hT", [D, S], BF16, kind="ExternalInput").ap()
    wq = nc.dram_tensor("wq", [D, HQ * DH], BF16, kind="ExternalInput").ap()
    wk = nc.dram_tensor("wk", [D, DH], BF16, kind="ExternalInput").ap()
    wv = nc.dram_tensor("wv", [D, DH], BF16, kind="ExternalInput").ap()
    wo = nc.dram_tensor("wo", [HQ * DH, D], BF16, kind="ExternalInput").ap()
    cosT = nc.dram_tensor("cosT", [DH, S], BF16, kind="ExternalInput").ap()
    sinT = nc.dram_tensor("sinT", [DH, S], BF16, kind="ExternalInput").ap()
    stair = nc.dram_tensor("stair", [128, 128], BF16,
                           kind="ExternalInput").ap()
    cstb = nc.dram_tensor("cstb", [128, 256], BF16, kind="ExternalInput").ap()
    onesf = nc.dram_tensor("onesf", [128, 128], F32R, kind="ExternalInput").ap()
    outT = nc.dram_tensor("outT", [D, S], BF16, kind="ExternalOutput").ap()

    with tile.TileContext(nc) as tc, \
            tc.tile_pool(name="main", bufs=1) as pm, \
            tc.tile_pool(name="hstream", bufs=5) as hp, \
            tc.tile_pool(name="expp", bufs=8) as ep, \
            tc.tile_pool(name="ropet", bufs=3) as rtp, \
            tc.tile_pool(name="accp", bufs=2) as ap_, \
            tc.tile_pool(name="dnp", bufs=2) as dp, \
            tc.tile_pool(name="ocp", bufs=6) as ocp, \
            tc.tile_pool(name="psA", bufs=3, space="PSUM") as psA, \
            tc.tile_pool(name="psS", bufs=2, space="PSUM") as psS, \
            tc.tile_pool(name="psO", bufs=1, space="PSUM") as psO, \
            tc.tile_pool(name="psC", bufs=2, space="PSUM") as psC:
        # long-lived SBUF tensors
        qT = [pm.tile([128, S], BF16, tag=f"qT{g}", name=f"qT{g}")
              for g in range(HQ)]
        kT = pm.tile([128, S], BF16, tag="kT")
        vT = pm.tile([128, S], BF16, tag="vT")
        v_all = pm.tile([128, S // 128, DH], BF16, tag="vall")
        o_attn = [pm.tile([128, S], BF16, tag=f"oT{g}", name=f"oT{g}")
                  for g in range(HQ)]
        cos_sb = pm.tile([128, S], BF16, tag="cos")
        sin_sb = pm.tile([128, S], BF16, tag="sin")
        stair_sb = pm.tile([128, 128], BF16, tag="stair")
        cst_sb = pm.tile([128, 256], BF16, tag="cstb")
        ones_sb = pm.tile([128, 128], F32R, tag="ones")
        wq_sb = pm.tile([128, DT, HQ * DH], BF16, tag="wq")
        wk_sb = pm.tile([128, DT, DH], BF16, tag="wk")
        wv_sb = pm.tile([128, DT, DH], BF16, tag="wv")
        wo_sb = pm.tile([128, HQ, D], BF16, tag="wo")

        rot_sb = cst_sb[:, 0:128]
        ident = cst_sb[:, 128:256]

        # ---- prologue DMAs (scalar ring; wk/wq first so MM 0 starts ~1us) ----
        wq_r = wq.rearrange("(t p) m -> p t m", p=128)
        wk_r = wk.rearrange("(t p) m -> p t m", p=128)
        wv_r = wv.rearrange("(t p) m -> p t m", p=128)
        # Startup is HBM-bandwidth-bound: pass1(0) needs wk + the LOW half
        # of wq (heads 0,1) + the hT slice (~7MB in ~21us). Defer everything
        # pass2 needs (wq high half, wv, RoPE tables) into the pass2 window,
        # which has DMA headroom.
        chunks = [(0, 1), (1, 3), (3, 7), (7, 15), (15, 24), (24, 32)]
        for c0, c1 in chunks:
            cs = slice(c0, c1)
            nc.scalar.dma_start(out=wk_sb[:, cs, :], in_=wk_r[:, cs, :])
            nc.scalar.dma_start(out=wq_sb[:, cs, 0:2 * DH],
                                in_=wq_r[:, cs, 0:2 * DH])
        for c0, c1 in chunks[:4]:
            cs = slice(c0, c1)
            nc.scalar.dma_start(out=wv_sb[:, cs, :], in_=wv_r[:, cs, :])
            nc.scalar.dma_start(out=wq_sb[:, cs, 2 * DH:4 * DH],
                                in_=wq_r[:, cs, 2 * DH:4 * DH])
        nc.scalar.dma_start(out=cst_sb, in_=cstb)
        nc.scalar.dma_start(out=cos_sb, in_=cosT)
        nc.scalar.dma_start(out=sin_sb, in_=sinT)
        for c0, c1 in chunks[4:]:
            cs = slice(c0, c1)
            nc.scalar.dma_start(out=wv_sb[:, cs, :], in_=wv_r[:, cs, :])
            nc.scalar.dma_start(out=wq_sb[:, cs, 2 * DH:4 * DH],
                                in_=wq_r[:, cs, 2 * DH:4 * DH])
        nc.scalar.dma_start(out=stair_sb, in_=stair)
        nc.scalar.dma_start(out=ones_sb, in_=onesf)

        hT_r = hT.rearrange("(t p) s -> p t s", p=128)

        # ---------------- emitters ----------------
        def emit_rope(blk, ssl):
            """blk[:, ssl] = blk*cos + rot(blk)*sin, rot via PE matmul."""
            pr = psS.tile([128, QSL], F32, tag="ps", name="pr")
            nc.tensor.matmul(pr, rot_sb, blk[:, ssl], start=True, stop=True)
            rs = rtp.tile([128, QSL], BF16, tag="rs")
            nc.vector.tensor_mul(rs, pr, sin_sb[:, ssl])
            cc = rtp.tile([128, QSL], BF16, tag="rc")
            nc.vector.tensor_mul(cc, blk[:, ssl], cos_sb[:, ssl])
            nc.vector.tensor_add(blk[:, ssl], cc, rs)

        def emit_vtr(t):
            """v_all[:, t, :] = vT[:, t*128:(t+1)*128].T (natural layout)."""
            pt = psS.tile([128, 128], BF16, tag="ps", name="pt")
            nc.tensor.transpose(pt, vT[:, t * 128:(t + 1) * 128], ident)
            nc.scalar.copy(v_all[:, t, :], pt)

        def emit_block(qs, Dt, pool=None):
            """One o_proj [128, QSL] output block for q-slice qs."""
            qsl_ = slice(qs * QSL, (qs + 1) * QSL)
            pool = pool if pool is not None else psC
            pf = pool.tile([128, QSL], F32,
                           tag="psA" if pool is psA else "psC", name="psC")
            for g in range(HQ):
                nc.tensor.matmul(
                    pf, wo_sb[:, g, Dt * 128:(Dt + 1) * 128],
                    o_attn[g][:, qsl_],
                    start=(g == 0), stop=(g == HQ - 1))
            oc = ocp.tile([128, QSL], BF16, tag="oc")
            nc.scalar.copy(oc, pf)
            nc.sync.dma_start(out=outT[Dt * 128:(Dt + 1) * 128, qsl_], in_=oc)

        def oproj_stream(qs, dts):
            for Dt in dts:
                emit_block(qs, Dt)
                yield

        def attn_stream(qs):
            """Head-serial flash attention for q-slice qs. One PV PSUM bank.
            Yields after each PE-op-sized step."""
            qb = qs * QSL
            qsl_ = slice(qb, qb + QSL)
            nkt = (qs + 1) * KPS
            for g in range(HQ):
                po = psO.tile([128, QSL], F32, tag="psO", name="po")
                acc = ap_.tile([128, QSL], F32R, tag="acc", name="acc")
                ex = [None] * nkt

                def emit_s(t):
                    delta = t * 128 - qb
                    dlo = max(0, delta)
                    ps = psS.tile([128, QSL], F32, tag="ps", name="ps")
                    nc.tensor.matmul(ps[:, dlo:],
                                     kT[:, t * 128:(t + 1) * 128],
                                     qT[g][:, qb + dlo:qb + QSL],
                                     start=True, stop=True)
                    e = ep.tile([128, QSL], BF16, tag="exp", name="ex")
                    nc.scalar.activation(e[:, dlo:], ps[:, dlo:], Exp,
                                         bias=0.0, scale=scale)
                    if delta >= 0:
                        # causal staircase on the 128 diagonal columns
                        nc.vector.tensor_mul(e[:, dlo:dlo + 128],
                                             e[:, dlo:dlo + 128], stair_sb)
                    ex[t] = e

                def emit_p(t):
                    dlo = max(0, t * 128 - qb)
                    e = ex[t]
                    nc.tensor.matmul(po[:, dlo:], v_all[:, t, :], e[:, dlo:],
                                     start=(t == 0), stop=(t == nkt - 1))
                    if t == 0:
                        nc.vector.tensor_copy(acc, e)
                    else:
                        nc.vector.tensor_add(acc[:, dlo:], acc[:, dlo:],
                                             e[:, dlo:])
                    ex[t] = None

                emit_s(0)
                yield
                for t in range(1, nkt):
                    emit_s(t)
                    yield
                    emit_p(t - 1)
                    yield
                emit_p(nkt - 1)
                yield
                # denominator (partition-sum via ones matmul) + normalize
                pd = psS.tile([128, QSL], F32, tag="ps", name="pd")
                nc.tensor.matmul(pd, ones_sb, acc, start=True, stop=True)
                dn = dp.tile([128, QSL], F32, tag="dn")
                nc.vector.reciprocal_approx_fast(dn, pd)
                nc.vector.tensor_mul(o_attn[g][:, qsl_], po, dn)
                yield

        def attn_count(qs):
            return HQ * (2 * (qs + 1) * KPS + 1)

        def merged(streams):
            """Proportionally interleave step-generators (Bresenham)."""
            counts = [n for _, n in streams]
            total = sum(counts)
            accs = [0.0] * len(streams)
            live = [g for g, _ in streams]
            for _ in range(total):
                for j in range(len(streams)):
                    accs[j] += counts[j]
                j = max(range(len(streams)), key=lambda k: accs[k])
                accs[j] -= total
                if next(live[j], _DONE) is not _DONE:
                    yield

        def gen_of(closures):
            for c in closures:
                c()
                yield

        # group selectors: (weight_tile_fn, dst, copy_engine)
        def pass_groups(half):
            if half == 0:
                return [
                    (lambda d: wk_sb[:, d, :], kT, 's'),
                    (lambda d: wq_sb[:, d, 0 * DH:1 * DH], qT[0], 'v'),
                    (lambda d: wq_sb[:, d, 1 * DH:2 * DH], qT[1], 's'),
                ]
            return [
                (lambda d: wv_sb[:, d, :], vT, 'v'),
                (lambda d: wq_sb[:, d, 2 * DH:3 * DH], qT[2], 's'),
                (lambda d: wq_sb[:, d, 3 * DH:4 * DH], qT[3], 'v'),
            ]

        _pref = [None]  # prefetched sub0 hT tile for the next slice

        def super_iter(sl, extras_it, n_extras):
            ssl = slice(sl * QSL, (sl + 1) * QSL)
            per_step = n_extras / 64.0
            budget = 0.0

            def pop():
                nonlocal budget
                budget += per_step
                while budget >= 1.0:
                    budget -= 1.0
                    if next(extras_it, _DONE) is _DONE:
                        budget = 0.0
                        return

            # hT slice staged once in SBUF sub-tiles; pass2 re-reads the
            # resident tiles (no HBM re-stream). Slice 0 uses fine-grained
            # leading blocks (the very first matmul then waits on ~224KB,
            # not 1.2MB, during the DMA spin-up); sub0 of the NEXT slice is
            # prefetched during pass2 so super-iteration boundaries don't
            # stall on DMA.
            if sl == 0:
                blocks = [(0, 1), (1, 2), (2, 4), (4, 8),
                          (8, 16), (16, 24), (24, 32)]
            else:
                blocks = [(0, 8), (8, 16), (16, 24), (24, 32)]
            btile = [None] * len(blocks)
            if _pref[0] is not None:
                btile[0] = _pref[0]
                _pref[0] = None

            def ht_op(d):
                for bi, (lo, hi) in enumerate(blocks):
                    if lo <= d < hi:
                        return btile[bi][:, d - lo, :]
                raise AssertionError

            for half in range(2):
                groups = pass_groups(half)
                pss = [psA.tile([128, QSL], F32, tag="psA",
                                name=f"psA_{sl}_{half}_{i}")
                       for i in range(3)]
                for d0 in range(0, DT, 2):
                    if half == 0:
                        for bi, (lo, hi) in enumerate(blocks):
                            if btile[bi] is None and lo <= d0 + 1:
                                if sl == 0 and hi - lo < 4:
                                    btile[bi] = pm.tile(
                                        [128, hi - lo, QSL], BF16,
                                        tag=f"hts0_{bi}", name=f"hts0_{bi}")
                                else:
                                    btile[bi] = hp.tile(
                                        [128, 8, QSL], BF16, tag="ht",
                                        name=f"ht_{sl}_{bi}")
                                nc.sync.dma_start(
                                    out=btile[bi][:, 0:hi - lo, :],
                                    in_=hT_r[:, lo:hi, ssl])
                    if half == 1 and d0 == 2 and sl + 1 < NQS:
                        nsl = slice((sl + 1) * QSL, (sl + 2) * QSL)
                        pt_ = hp.tile([128, 8, QSL], BF16, tag="ht",
                                      name=f"ht_{sl + 1}_0")
                        nc.sync.dma_start(out=pt_, in_=hT_r[:, 0:8, nsl])
                        _pref[0] = pt_
                    for j in range(2):
                        d = d0 + j
                        for gi, (wsel, _, _) in enumerate(groups):
                            nc.tensor.matmul(pss[gi], wsel(d), ht_op(d),
                                             start=(d == 0),
                                             stop=(d == DT - 1))
                        pop()
                for gi, (_, dst, eng) in enumerate(groups):
                    if eng == 's':
                        nc.scalar.copy(dst[:, ssl], pss[gi])
                    else:
                        nc.vector.tensor_copy(dst[:, ssl], pss[gi])
                if half == 0:
                    # k/q0/q1 of this slice exist now; RoPE them so the
                    # next super-iteration's attention can use them.
                    for blk in (kT, qT[0], qT[1]):
                        emit_rope(blk, ssl)

        # ---------------- main pipeline ----------------
        for sl in range(NQS):
            pre = []
            streams = []
            if sl >= 1:
                pssl = slice((sl - 1) * QSL, sl * QSL)
                pre.append(lambda s_=pssl: emit_rope(qT[2], s_))
                pre.append(lambda s_=pssl: emit_rope(qT[3], s_))
                for tt in range(KPS):
                    pre.append(lambda t_=(sl - 1) * KPS + tt: emit_vtr(t_))
                streams.append((attn_stream(sl - 1), attn_count(sl - 1)))
            if sl >= 2:
                streams.append((oproj_stream(sl - 2, range(NDT)), NDT))
            extras = _chain(gen_of(pre), merged(streams))
            n_extras = len(pre) + sum(n for _, n in streams)
            super_iter(sl, extras, n_extras)
            for _ in extras:   # drain leftovers (rounding)
                pass
            if sl == 0:
                # wo needed from first o_proj block (~super-iter 2);
                # deferred + chunked to keep it off the critical startup DMAs.
                wo_r = wo.rearrange("(t p) n -> p t n", p=128)
                for t_ in range(HQ):
                    nc.scalar.dma_start(out=wo_sb[:, t_, :],
                                        in_=wo_r[:, t_, :])

        # ---------------- tail: attn(3) + o_proj(2,3) ----------------
        lsl = slice((NQS - 1) * QSL, NQS * QSL)
        emit_rope(qT[2], lsl)
        emit_rope(qT[3], lsl)
        for tt in range(KPS):
            emit_vtr((NQS - 1) * KPS + tt)
        reserve = 4
        tail = merged([(attn_stream(NQS - 1), attn_count(NQS - 1)),
                       (oproj_stream(NQS - 2, range(NDT - reserve)),
                        NDT - reserve)])
        for _ in tail:
            pass
        # reserve blocks cover the last head's denominator-chain latency;
        # the final chain borrows the now-idle psA pool for a deeper
        # PSUM pipeline (3 bufs vs 2).
        for Dt in range(NDT - reserve, NDT):
            emit_block(NQS - 2, Dt, pool=psA)
        for Dt in range(NDT):
            emit_block(NQS - 1, Dt, pool=psA if Dt % 2 else psC)

    nc.compile()
    return nc


def _chain(*its):
    for it in its:
        yield from it


def make_tables(cfg: Cfg, position_ids: np.ndarray):
    """cosT/sinT [128, S]: row d holds cos/sin(pos * invfreq[d % 64])."""
    half = cfg.DH // 2
    inv = 1.0 / (cfg.theta ** (np.arange(half, dtype=np.float64) * 2.0 / cfg.DH))
    pos = np.asarray(position_ids).reshape(-1).astype(np.float64)  # [S]
    ang = inv[:, None] * pos[None, :]                              # [64, S]
    cosT = np.concatenate([np.cos(ang), np.cos(ang)], 0)
    sinT = np.concatenate([np.sin(ang), np.sin(ang)], 0)
    return cosT, sinT


def make_cst(cfg: Cfg):
    """[128, 256] cols 0-127: rotate-half stationary matrix (out = M^T @ x,
    out[:64] = -x[64:], out[64:] = x[:64]); cols 128-255: identity."""
    half = cfg.DH // 2
    m = np.zeros((128, 256), np.float64)
    for i in range(half):
        m[i + half, i] = -1.0
        m[i, i + half] = 1.0
    m[:, 128:256] = np.eye(128)
    return m


def make_stair():
    """0/1 staircase [128,128]: col j, row p -> 1 if j >= p else 0.
    Applied to the 128 diagonal columns [dlo, dlo+128) of each diagonal
    score tile (delta >= 0), where col j-dlo vs row p encodes k <= q."""
    j = np.arange(128)[None, :]
    p = np.arange(128)[:, None]
    return (j >= p).astype(np.float64)


_cache = threading.Lock()
_nc_full = None


def _get_nc():
    global _nc_full
    with _cache:
        if _nc_full is None:
            _nc_full = build_nc(FULL)
    return _nc_full


def core_inputs(cfg: Cfg, c: int, position_ids, hidden_states, Wq, Wk, Wv, Wo):
    """Build the per-core input map (numpy, bf16 operands) for core c."""
    from ml_dtypes import bfloat16

    def bf(x):
        return np.ascontiguousarray(np.asarray(x).astype(bfloat16))

    S, D, HQ, DH = cfg.S, cfg.D, cfg.HQ, cfg.DH
    hT = np.asarray(hidden_states, dtype=np.float32).reshape(S, D).T
    cosT, sinT = make_tables(cfg, position_ids)
    qc = slice(c * HQ * DH, (c + 1) * HQ * DH)
    kc = slice(c * DH, (c + 1) * DH)
    return {
        "hT": bf(hT),
        "wq": bf(np.asarray(Wq, np.float32)[:, qc]),
        "wk": bf(np.asarray(Wk, np.float32)[:, kc]),
        "wv": bf(np.asarray(Wv, np.float32)[:, kc]),
        "wo": bf(np.asarray(Wo, np.float32)[qc, :]),
        "cosT": bf(cosT),
        "sinT": bf(sinT),
        "stair": bf(make_stair()),
        "cstb": bf(make_cst(cfg)),
        "onesf": np.ones((128, 128), np.float32),
    }


def kernel(position_ids, hidden_states, Wq, Wk, Wv, Wo, _trace=False):
    from concourse.bass_utils import run_bass_kernel_spmd

    cfg = FULL
    nc = _get_nc()
    args = (position_ids, hidden_states, Wq, Wk, Wv, Wo)
    in_maps = [core_inputs(cfg, c, *args) for c in range(cfg.cores)]
    res = run_bass_kernel_spmd(nc, in_maps, core_ids=list(range(cfg.cores)),
                               trace=_trace)
    out = np.zeros((cfg.S, cfg.D), np.float64)
    for c in range(cfg.cores):
        out += res.results[c]["outT"].T.astype(np.float64)
    ret = out.astype(np.float32).reshape(1, cfg.S, cfg.D)
    if _trace:
        return ret, res
    return ret


# revision 23
# speedup vs baseline: 1.0639x; 1.0238x over previous
"""Trainium2 Bass kernel for Llama-style GQA attention block (B=1, S=2048,
D=4096, 32 q heads / 8 kv heads, head_dim 128, neox RoPE, causal).

Sharding: tensor-parallel over kv heads across 8 NeuronCores. Core c gets
kv head c and q heads [4c, 4c+4). Each core computes a full [S, D] partial
of the output (o_proj row-parallel); host sums the 8 partials.

v3 (fused single pipeline): projections, attention, and o_proj emitted as
one interleaved PE instruction stream so the tensor engine never idles at
phase boundaries (the v2 A->B boundary cost ~5.5us idle + a HAM re-throttle).

Structure per S-slice sl of 512 (super-iteration):
  - projections run in TWO passes of 3 groups each (pass1 = {k, q0, q1},
    pass2 = {v, q2, q3}) so they hold only 3 PSUM banks; hT is re-streamed
    from HBM for pass2 (DMA has headroom, PSUM does not).
  - attention for q-slice qs=sl-1 runs HEAD-SERIAL (one PSUM bank for the
    PV accumulator), its score/PV matmuls popped between projection d-steps;
    exp latency hidden by a 2-deep score pipeline + proj-MM filler.
  - o_proj blocks of qs=sl-2 are popped as additional PE filler.
  PSUM budget: 3 (proj passes) + 2 (scores/rope/transpose/denominator,
  tag-shared) + 1 (PV accum) + 2 (o_proj) = 8 banks exactly.

Other changes vs v2: PV matmuls, exps and denominator adds are trimmed to
[dlo:] on diagonal tiles (no stale-byte priming needed; the causal staircase
mask shrinks to a single [128,128] 0/1 multiply), the denominator accumulator
is f32r from the start, and outT is stored bf16 (halves output DMA).
"""

import threading
from dataclasses import dataclass

import numpy as np


@dataclass(frozen=True)
class Cfg:
    S: int = 2048      # sequence length
    D: int = 4096      # hidden size
    HQ: int = 4        # q heads per core
    DH: int = 128      # head dim
    QSL: int = 512     # q-slice width (= matmul N)
    theta: float = 10000.0
    cores: int = 8


FULL = Cfg()
_DONE = object()


def build_nc(cfg: Cfg):
    import concourse.bass as bass  # noqa: F401
    import concourse.mybir as mybir
    import concourse.tile as tile
    from concourse import bacc

    F32 = mybir.dt.float32
    F32R = mybir.dt.float32r
    BF16 = mybir.dt.bfloat16

    S, D, HQ, DH, QSL = cfg.S, cfg.D, cfg.HQ, cfg.DH, cfg.QSL
    DT = D // 128          # d (contraction) tiles
    NQS = S // QSL         # q slices / S slices
    NDT = D // 128         # output D row-tiles (o_proj)
    KPS = QSL // 128       # k-tiles per slice
    scale = float(DH) ** -0.5
    Exp = mybir.ActivationFunctionType.Exp

    nc = bacc.Bacc("TRN2", target_bir_lowering=False, debug=False,
                   num_devices=cfg.cores)

    # All inputs host-packed to [128 partitions, ...] with per-partition
    # contiguous payloads so every DMA lowers to one large descriptor per
    # partition (512B-segment transfers were descriptor-rate-bound).
    hTp = nc.dram_tensor("hTp", [128, NQS * DT, QSL], BF16,
                         kind="ExternalInput").ap()
    wqlo = nc.dram_tensor("wqlo", [128, DT, 2 * DH], BF16,
                          kind="ExternalInput").ap()
    wqhi = nc.dram_tensor("wqhi", [128, DT, 2 * DH], BF16,
                          kind="ExternalInput").ap()
    wk = nc.dram_tensor("wk", [128, DT, DH], BF16, kind="ExternalInput").ap()
    wv = nc.dram_tensor("wv", [128, DT, DH], BF16, kind="ExternalInput").ap()
    wo = nc.dram_tensor("wo", [128, HQ, D], BF16, kind="ExternalInput").ap()
    cosT = nc.dram_tensor("cosT", [DH, S], BF16, kind="ExternalInput").ap()
    sinT = nc.dram_tensor("sinT", [DH, S], BF16, kind="ExternalInput").ap()
    stair = nc.dram_tensor("stair", [128, 128], BF16,
                           kind="ExternalInput").ap()
    cstb = nc.dram_tensor("cstb", [128, 256], BF16, kind="ExternalInput").ap()
    onesf = nc.dram_tensor("onesf", [128, 128], F32R, kind="ExternalInput").ap()
    outT = nc.dram_tensor("outT", [D, S], BF16, kind="ExternalOutput").ap()

    with tile.TileContext(nc) as tc, \
            tc.tile_pool(name="main", bufs=1) as pm, \
            tc.tile_pool(name="hstream", bufs=5) as hp, \
            tc.tile_pool(name="expp", bufs=8) as ep, \
            tc.tile_pool(name="ropet", bufs=3) as rtp, \
            tc.tile_pool(name="accp", bufs=2) as ap_, \
            tc.tile_pool(name="dnp", bufs=2) as dp, \
            tc.tile_pool(name="ocp", bufs=6) as ocp, \
            tc.tile_pool(name="psA", bufs=3, space="PSUM") as psA, \
            tc.tile_pool(name="psS", bufs=2, space="PSUM") as psS, \
            tc.tile_pool(name="psO", bufs=1, space="PSUM") as psO, \
            tc.tile_pool(name="psC", bufs=2, space="PSUM") as psC:
        # long-lived SBUF tensors
        qT = [pm.tile([128, S], BF16, tag=f"qT{g}", name=f"qT{g}")
              for g in range(HQ)]
        kT = pm.tile([128, S], BF16, tag="kT")
        vT = pm.tile([128, S], BF16, tag="vT")
        v_all = pm.tile([128, S // 128, DH], BF16, tag="vall")
        o_attn = [pm.tile([128, S], BF16, tag=f"oT{g}", name=f"oT{g}")
                  for g in range(HQ)]
        cos_sb = pm.tile([128, S], BF16, tag="cos")
        sin_sb = pm.tile([128, S], BF16, tag="sin")
        stair_sb = pm.tile([128, 128], BF16, tag="stair")
        cst_sb = pm.tile([128, 256], BF16, tag="cstb")
        ones_sb = pm.tile([128, 128], F32R, tag="ones")
        wqlo_sb = pm.tile([128, DT, 2 * DH], BF16, tag="wqlo")
        wqhi_sb = pm.tile([128, DT, 2 * DH], BF16, tag="wqhi")
        wk_sb = pm.tile([128, DT, DH], BF16, tag="wk")
        wv_sb = pm.tile([128, DT, DH], BF16, tag="wv")
        wo_sb = pm.tile([128, HQ, D], BF16, tag="wo")

        rot_sb = cst_sb[:, 0:128]
        ident = cst_sb[:, 128:256]

        # ---- prologue DMAs (scalar ring; wk/wq first so MM 0 starts ~1us) ----
        # Startup is HBM-bandwidth-bound: pass1(0) needs wk + the LOW half
        # of wq (heads 0,1) + the hT slice (~7MB in ~21us). Defer everything
        # pass2 needs (wq high half, wv, RoPE tables) into the pass2 window,
        # which has DMA headroom.
        chunks = [(0, 1), (1, 3), (3, 7), (7, 15), (15, 24), (24, 32)]
        for c0, c1 in chunks:
            cs = slice(c0, c1)
            nc.scalar.dma_start(out=wk_sb[:, cs, :], in_=wk[:, cs, :])
            nc.scalar.dma_start(out=wqlo_sb[:, cs, :], in_=wqlo[:, cs, :])
        for c0, c1 in chunks[:4]:
            cs = slice(c0, c1)
            nc.scalar.dma_start(out=wv_sb[:, cs, :], in_=wv[:, cs, :])
            nc.scalar.dma_start(out=wqhi_sb[:, cs, :], in_=wqhi[:, cs, :])
        nc.scalar.dma_start(out=cst_sb, in_=cstb)
        nc.scalar.dma_start(out=cos_sb, in_=cosT)
        nc.scalar.dma_start(out=sin_sb, in_=sinT)
        for c0, c1 in chunks[4:]:
            cs = slice(c0, c1)
            nc.scalar.dma_start(out=wv_sb[:, cs, :], in_=wv[:, cs, :])
            nc.scalar.dma_start(out=wqhi_sb[:, cs, :], in_=wqhi[:, cs, :])
        nc.scalar.dma_start(out=stair_sb, in_=stair)
        nc.scalar.dma_start(out=ones_sb, in_=onesf)


        # ---------------- emitters ----------------
        def emit_rope(blk, ssl):
            """blk[:, ssl] = blk*cos + rot(blk)*sin, rot via PE matmul."""
            pr = psS.tile([128, QSL], F32, tag="ps", name="pr")
            nc.tensor.matmul(pr, rot_sb, blk[:, ssl], start=True, stop=True)
            rs = rtp.tile([128, QSL], BF16, tag="rs")
            nc.vector.tensor_mul(rs, pr, sin_sb[:, ssl])
            cc = rtp.tile([128, QSL], BF16, tag="rc")
            nc.vector.tensor_mul(cc, blk[:, ssl], cos_sb[:, ssl])
            nc.vector.tensor_add(blk[:, ssl], cc, rs)

        def emit_vtr(t):
            """v_all[:, t, :] = vT[:, t*128:(t+1)*128].T (natural layout)."""
            pt = psS.tile([128, 128], BF16, tag="ps", name="pt")
            nc.tensor.transpose(pt, vT[:, t * 128:(t + 1) * 128], ident)
            nc.scalar.copy(v_all[:, t, :], pt)

        def emit_block(qs, Dt, pool=None):
            """One o_proj [128, QSL] output block for q-slice qs."""
            qsl_ = slice(qs * QSL, (qs + 1) * QSL)
            pool = pool if pool is not None else psC
            pf = pool.tile([128, QSL], F32,
                           tag="psA" if pool is psA else "psC", name="psC")
            for g in range(HQ):
                nc.tensor.matmul(
                    pf, wo_sb[:, g, Dt * 128:(Dt + 1) * 128],
                    o_attn[g][:, qsl_],
                    start=(g == 0), stop=(g == HQ - 1))
            oc = ocp.tile([128, QSL], BF16, tag="oc")
            nc.scalar.copy(oc, pf)
            nc.sync.dma_start(out=outT[Dt * 128:(Dt + 1) * 128, qsl_], in_=oc)

        def oproj_stream(qs, dts):
            for Dt in dts:
                emit_block(qs, Dt)
                yield

        def attn_stream(qs):
            """Head-serial flash attention for q-slice qs. One PV PSUM bank.
            Yields after each PE-op-sized step."""
            qb = qs * QSL
            qsl_ = slice(qb, qb + QSL)
            nkt = (qs + 1) * KPS
            for g in range(HQ):
                po = psO.tile([128, QSL], F32, tag="psO", name="po")
                acc = ap_.tile([128, QSL], F32R, tag="acc", name="acc")
                ex = [None] * nkt

                def emit_s(t):
                    delta = t * 128 - qb
                    dlo = max(0, delta)
                    ps = psS.tile([128, QSL], F32, tag="ps", name="ps")
                    nc.tensor.matmul(ps[:, dlo:],
                                     kT[:, t * 128:(t + 1) * 128],
                                     qT[g][:, qb + dlo:qb + QSL],
                                     start=True, stop=True)
                    e = ep.tile([128, QSL], BF16, tag="exp", name="ex")
                    nc.scalar.activation(e[:, dlo:], ps[:, dlo:], Exp,
                                         bias=0.0, scale=scale)
                    if delta >= 0:
                        # causal staircase on the 128 diagonal columns
                        nc.vector.tensor_mul(e[:, dlo:dlo + 128],
                                             e[:, dlo:dlo + 128], stair_sb)
                    ex[t] = e

                def emit_p(t):
                    dlo = max(0, t * 128 - qb)
                    e = ex[t]
                    nc.tensor.matmul(po[:, dlo:], v_all[:, t, :], e[:, dlo:],
                                     start=(t == 0), stop=(t == nkt - 1))
                    if t == 0:
                        nc.vector.tensor_copy(acc, e)
                    else:
                        nc.vector.tensor_add(acc[:, dlo:], acc[:, dlo:],
                                             e[:, dlo:])
                    ex[t] = None

                emit_s(0)
                yield
                for t in range(1, nkt):
                    emit_s(t)
                    yield
                    emit_p(t - 1)
                    yield
                emit_p(nkt - 1)
                yield
                # denominator (partition-sum via ones matmul) + normalize
                pd = psS.tile([128, QSL], F32, tag="ps", name="pd")
                nc.tensor.matmul(pd, ones_sb, acc, start=True, stop=True)
                dn = dp.tile([128, QSL], F32, tag="dn")
                nc.vector.reciprocal_approx_fast(dn, pd)
                nc.vector.tensor_mul(o_attn[g][:, qsl_], po, dn)
                yield

        def attn_count(qs):
            return HQ * (2 * (qs + 1) * KPS + 1)

        def merged(streams):
            """Proportionally interleave step-generators (Bresenham)."""
            counts = [n for _, n in streams]
            total = sum(counts)
            accs = [0.0] * len(streams)
            live = [g for g, _ in streams]
            for _ in range(total):
                for j in range(len(streams)):
                    accs[j] += counts[j]
                j = max(range(len(streams)), key=lambda k: accs[k])
                accs[j] -= total
                if next(live[j], _DONE) is not _DONE:
                    yield

        def gen_of(closures):
            for c in closures:
                c()
                yield

        # group selectors: (weight_tile_fn, dst, copy_engine)
        def pass_groups(half):
            if half == 0:
                return [
                    (lambda d: wk_sb[:, d, :], kT, 's'),
                    (lambda d: wqlo_sb[:, d, 0:DH], qT[0], 'v'),
                    (lambda d: wqlo_sb[:, d, DH:2 * DH], qT[1], 's'),
                ]
            return [
                (lambda d: wv_sb[:, d, :], vT, 'v'),
                (lambda d: wqhi_sb[:, d, 0:DH], qT[2], 's'),
                (lambda d: wqhi_sb[:, d, DH:2 * DH], qT[3], 'v'),
            ]

        _pref = [None]  # prefetched sub0 hT tile for the next slice

        def super_iter(sl, extras_it, n_extras):
            ssl = slice(sl * QSL, (sl + 1) * QSL)
            per_step = n_extras / 64.0
            budget = 0.0

            def pop():
                nonlocal budget
                budget += per_step
                while budget >= 1.0:
                    budget -= 1.0
                    if next(extras_it, _DONE) is _DONE:
                        budget = 0.0
                        return

            # hT slice staged once in SBUF sub-tiles; pass2 re-reads the
            # resident tiles (no HBM re-stream). Slice 0 uses fine-grained
            # leading blocks (the very first matmul then waits on ~224KB,
            # not 1.2MB, during the DMA spin-up); sub0 of the NEXT slice is
            # prefetched during pass2 so super-iteration boundaries don't
            # stall on DMA.
            if sl == 0:
                blocks = [(0, 1), (1, 2), (2, 4), (4, 8),
                          (8, 16), (16, 24), (24, 32)]
            else:
                blocks = [(0, 8), (8, 16), (16, 24), (24, 32)]
            btile = [None] * len(blocks)
            if _pref[0] is not None:
                btile[0] = _pref[0]
                _pref[0] = None

            def ht_op(d):
                for bi, (lo, hi) in enumerate(blocks):
                    if lo <= d < hi:
                        return btile[bi][:, d - lo, :]
                raise AssertionError

            for half in range(2):
                groups = pass_groups(half)
                pss = [psA.tile([128, QSL], F32, tag="psA",
                                name=f"psA_{sl}_{half}_{i}")
                       for i in range(3)]
                for d0 in range(0, DT, 2):
                    if half == 0:
                        for bi, (lo, hi) in enumerate(blocks):
                            if btile[bi] is None and lo <= d0 + 1:
                                if sl == 0 and hi - lo < 4:
                                    btile[bi] = pm.tile(
                                        [128, hi - lo, QSL], BF16,
                                        tag=f"hts0_{bi}", name=f"hts0_{bi}")
                                else:
                                    btile[bi] = hp.tile(
                                        [128, 8, QSL], BF16, tag="ht",
                                        name=f"ht_{sl}_{bi}")
                                nc.sync.dma_start(
                                    out=btile[bi][:, 0:hi - lo, :],
                                    in_=hTp[:, sl * DT + lo:sl * DT + hi, :])
                    if half == 1 and d0 == 2 and sl + 1 < NQS:
                        pt_ = hp.tile([128, 8, QSL], BF16, tag="ht",
                                      name=f"ht_{sl + 1}_0")
                        nc.sync.dma_start(
                            out=pt_, in_=hTp[:, (sl + 1) * DT:(sl + 1) * DT + 8, :])
                        _pref[0] = pt_
                    for j in range(2):
                        d = d0 + j
                        for gi, (wsel, _, _) in enumerate(groups):
                            nc.tensor.matmul(pss[gi], wsel(d), ht_op(d),
                                             start=(d == 0),
                                             stop=(d == DT - 1))
                        pop()
                for gi, (_, dst, eng) in enumerate(groups):
                    if eng == 's':
                        nc.scalar.copy(dst[:, ssl], pss[gi])
                    else:
                        nc.vector.tensor_copy(dst[:, ssl], pss[gi])
                if half == 0:
                    # k/q0/q1 of this slice exist now; RoPE them so the
                    # next super-iteration's attention can use them.
                    for blk in (kT, qT[0], qT[1]):
                        emit_rope(blk, ssl)

        # ---------------- main pipeline ----------------
        for sl in range(NQS):
            pre = []
            streams = []
            if sl >= 1:
                pssl = slice((sl - 1) * QSL, sl * QSL)
                pre.append(lambda s_=pssl: emit_rope(qT[2], s_))
                pre.append(lambda s_=pssl: emit_rope(qT[3], s_))
                for tt in range(KPS):
                    pre.append(lambda t_=(sl - 1) * KPS + tt: emit_vtr(t_))
                streams.append((attn_stream(sl - 1), attn_count(sl - 1)))
            if sl >= 2:
                streams.append((oproj_stream(sl - 2, range(NDT)), NDT))
            extras = _chain(gen_of(pre), merged(streams))
            n_extras = len(pre) + sum(n for _, n in streams)
            super_iter(sl, extras, n_extras)
            for _ in extras:   # drain leftovers (rounding)
                pass
            if sl == 0:
                # wo needed from first o_proj block (~super-iter 2);
                # deferred + chunked to keep it off the critical startup DMAs.
                for t_ in range(HQ):
                    nc.scalar.dma_start(out=wo_sb[:, t_, :],
                                        in_=wo[:, t_, :])

        # ---------------- tail: attn(3) + o_proj(2,3) ----------------
        lsl = slice((NQS - 1) * QSL, NQS * QSL)
        emit_rope(qT[2], lsl)
        emit_rope(qT[3], lsl)
        for tt in range(KPS):
            emit_vtr((NQS - 1) * KPS + tt)
        reserve = 4
        tail = merged([(attn_stream(NQS - 1), attn_count(NQS - 1)),
                       (oproj_stream(NQS - 2, range(NDT - reserve)),
                        NDT - reserve)])
        for _ in tail:
            pass
        # reserve blocks cover the last head's denominator-chain latency;
        # the final chain borrows the now-idle psA pool for a deeper
        # PSUM pipeline (3 bufs vs 2).
        for Dt in range(NDT - reserve, NDT):
            emit_block(NQS - 2, Dt, pool=psA)
        for Dt in range(NDT):
            emit_block(NQS - 1, Dt, pool=psA if Dt % 2 else psC)

    nc.compile()
    return nc


def _chain(*its):
    for it in its:
        yield from it


def make_tables(cfg: Cfg, position_ids: np.ndarray):
    """cosT/sinT [128, S]: row d holds cos/sin(pos * invfreq[d % 64])."""
    half = cfg.DH // 2
    inv = 1.0 / (cfg.theta ** (np.arange(half, dtype=np.float64) * 2.0 / cfg.DH))
    pos = np.asarray(position_ids).reshape(-1).astype(np.float64)  # [S]
    ang = inv[:, None] * pos[None, :]                              # [64, S]
    cosT = np.concatenate([np.cos(ang), np.cos(ang)], 0)
    sinT = np.concatenate([np.sin(ang), np.sin(ang)], 0)
    return cosT, sinT


def make_cst(cfg: Cfg):
    """[128, 256] cols 0-127: rotate-half stationary matrix (out = M^T @ x,
    out[:64] = -x[64:], out[64:] = x[:64]); cols 128-255: identity."""
    half = cfg.DH // 2
    m = np.zeros((128, 256), np.float64)
    for i in range(half):
        m[i + half, i] = -1.0
        m[i, i + half] = 1.0
    m[:, 128:256] = np.eye(128)
    return m


def make_stair():
    """0/1 staircase [128,128]: col j, row p -> 1 if j >= p else 0.
    Applied to the 128 diagonal columns [dlo, dlo+128) of each diagonal
    score tile (delta >= 0), where col j-dlo vs row p encodes k <= q."""
    j = np.arange(128)[None, :]
    p = np.arange(128)[:, None]
    return (j >= p).astype(np.float64)


_cache = threading.Lock()
_nc_full = None


def _get_nc():
    global _nc_full
    with _cache:
        if _nc_full is None:
            _nc_full = build_nc(FULL)
    return _nc_full


def _pack_w(w):
    """[D, M] -> [128, D//128, M]: row (p, t) = w[t*128 + p]. Per-partition
    payload is contiguous, so chunk DMAs lower to one descriptor/partition."""
    D, M = w.shape
    return w.reshape(D // 128, 128, M).transpose(1, 0, 2)


def core_inputs(cfg: Cfg, c: int, position_ids, hidden_states, Wq, Wk, Wv, Wo):
    """Build the per-core input map (numpy, bf16 operands) for core c."""
    from ml_dtypes import bfloat16

    def bf(x):
        return np.ascontiguousarray(np.asarray(x).astype(bfloat16))

    S, D, HQ, DH, QSL = cfg.S, cfg.D, cfg.HQ, cfg.DH, cfg.QSL
    DT, NQS = D // 128, S // QSL
    hT = np.asarray(hidden_states, dtype=np.float32).reshape(S, D).T
    # hTp[p, sl*DT + t, m] = hT[t*128 + p, sl*QSL + m]
    hTp = (hT.reshape(DT, 128, NQS, QSL).transpose(1, 2, 0, 3)
           .reshape(128, NQS * DT, QSL))
    cosT, sinT = make_tables(cfg, position_ids)
    qc = slice(c * HQ * DH, (c + 1) * HQ * DH)
    kc = slice(c * DH, (c + 1) * DH)
    wq_c = np.asarray(Wq, np.float32)[:, qc]
    # wo[p, g, n] = Wo[qc][g*128 + p, n]
    wo_c = np.asarray(Wo, np.float32)[qc, :].reshape(HQ, 128, D).transpose(1, 0, 2)
    return {
        "hTp": bf(hTp),
        "wqlo": bf(_pack_w(wq_c[:, 0:2 * DH])),
        "wqhi": bf(_pack_w(wq_c[:, 2 * DH:4 * DH])),
        "wk": bf(_pack_w(np.asarray(Wk, np.float32)[:, kc])),
        "wv": bf(_pack_w(np.asarray(Wv, np.float32)[:, kc])),
        "wo": bf(wo_c),
        "cosT": bf(cosT),
        "sinT": bf(sinT),
        "stair": bf(make_stair()),
        "cstb": bf(make_cst(cfg)),
        "onesf": np.ones((128, 128), np.float32),
    }


def kernel(position_ids, hidden_states, Wq, Wk, Wv, Wo, _trace=False):
    from concourse.bass_utils import run_bass_kernel_spmd

    cfg = FULL
    nc = _get_nc()
    args = (position_ids, hidden_states, Wq, Wk, Wv, Wo)
    in_maps = [core_inputs(cfg, c, *args) for c in range(cfg.cores)]
    res = run_bass_kernel_spmd(nc, in_maps, core_ids=list(range(cfg.cores)),
                               trace=_trace)
    out = np.zeros((cfg.S, cfg.D), np.float64)
    for c in range(cfg.cores):
        out += res.results[c]["outT"].T.astype(np.float64)
    ret = out.astype(np.float32).reshape(1, cfg.S, cfg.D)
    if _trace:
        return ret, res
    return ret
